# revision 7
# baseline (speedup 1.0000x reference)
"""TRN2 Bass kernel for nn_MultiHeadAttention_25598005084384.

Reference computation (B=4, S=1024, D_MODEL=1024, H=16, DEPTH=64, D_SP=512):
    sp   = relu(min_distance_matrix @ W_sp + b_sp)          [B,S,512]
    qh   = split_heads(q @ Wq + bq)                         [B,H,S,64]
    kh   = split_heads(sp @ Wk + bk)
    vh   = split_heads(sp @ Wv + bv)
    lg   = qh @ kh^T / 8 + mask * -1e9
    attn = softmax(lg)                                      [B,H,S,S]  (output 2)
    out  = (attn @ vh).merge_heads() @ Wo + bo              [B,S,1024] (output 1)

Sharding: 8 cores = (batch b in 0..3) x (query-chunk qc in 0..1, 512 queries).
Everything is computed in transposed ("T") layouts so the PE contraction axis
is always the SBUF partition axis and no on-device transposes are needed:
  spT[dsp,k'], khT/qhT[dm,.], vh natural [k',dm] head-interleaved with a ones
  column (so the PV matmul's row 64 yields softmax denominators), logits
  accumulated on top of a PSUM pre-primed with mask*-60000 (identity matmul),
  ACT exp evicts PSUM->SBUF, unnormalized masked exp goes out as attnT; host
  normalizes + transposes on unshard. outT = Wo^T @ concatT; host adds bo.

Precision: matmul operands in fp16 (10-bit mantissa, ~6e-4 end-to-end
scale-relative error) except the PV matmul which runs in float32r so the
attention-probability output is not further degraded. PSUM accumulates fp32.
fp16 runs at bf16 speed on the PE (1 cyc/row + fast weight load) vs 4x
slower for true fp32.
"""
import os
import sys

sys.path.insert(0, "/opt/trn_rl_repo")
import numpy as np
import concourse.bass as bass
import concourse.bacc as bacc
import concourse.tile as tile
from concourse import mybir
from concourse.bass_utils import run_bass_kernel_spmd

f32 = mybir.dt.float32
f32r = mybir.dt.float32r
f16 = mybir.dt.float16
AF = mybir.ActivationFunctionType
ALU = mybir.AluOpType

B, S, DM, H, DEPTH, DSP = 4, 1024, 1024, 16, 64, 512
Q = 512            # queries per core
NCORES = 8
KT = S // 128      # 8 key partition-tiles
W65 = DEPTH + 1    # vh head stride (64 cols + ones col)
MASKVAL = np.float32(-60000.0)   # fp16-representable; exp() underflows to 0

# stash of the last run's BassKernelResults for test harnesses
last_results = None


def _rnd_f32r(x):
    """Round-to-nearest fp32 -> 10-bit-mantissa (f32r) on host."""
    x = np.ascontiguousarray(x, np.float32)
    u = x.view(np.uint32)
    out = (((u.astype(np.uint64) + (1 << 12)) >> 13) << 13).astype(np.uint32)
    return out.view(np.float32)


def _build_nc():
    nc = bacc.Bacc(None, target_bir_lowering=False)

    # ---- DRAM I/O (per-core shard shapes) ----
    mdmT = nc.dram_tensor("mdmT", [S, S], f16, kind="ExternalInput")      # [s, k']
    qT = nc.dram_tensor("qT", [DM, Q], f16, kind="ExternalInput")         # [dm, q]
    maskT = nc.dram_tensor("maskT", [S, Q], f16, kind="ExternalInput")    # (1-mask).T
    W_sp = nc.dram_tensor("W_sp", [S, DSP], f16, kind="ExternalInput")
    Wq = nc.dram_tensor("Wq", [DM, DM], f16, kind="ExternalInput")        # pre-scaled /8
    Wk = nc.dram_tensor("Wk", [DSP, DM], f16, kind="ExternalInput")
    Wv = nc.dram_tensor("Wv", [DSP, DM], f16, kind="ExternalInput")
    Wo = nc.dram_tensor("Wo", [DM, DM], f16, kind="ExternalInput")
    b_sp = nc.dram_tensor("b_sp", [DSP], f32, kind="ExternalInput")
    bq = nc.dram_tensor("bq", [DM], f32, kind="ExternalInput")            # pre-scaled /8
    bk = nc.dram_tensor("bk", [DM], f32, kind="ExternalInput")
    bv = nc.dram_tensor("bv", [DM], f16, kind="ExternalInput")
    ones_row = nc.dram_tensor("ones_row", [1, 128], f16, kind="ExternalInput")

    attnT_d = nc.dram_tensor("attnT", [H, S, Q], f32r, kind="ExternalOutput")
    sums_d = nc.dram_tensor("sums", [H, Q], f32, kind="ExternalOutput")
    outT_d = nc.dram_tensor("outT", [DM, Q], f32, kind="ExternalOutput")

    with tile.TileContext(nc) as tc:
        with (
            tc.tile_pool(name="sb", bufs=1) as sb,
            tc.tile_pool(name="exps", bufs=3) as exps,
            tc.tile_pool(name="small", bufs=2) as small,
            tc.tile_pool(name="ps", bufs=2, space="PSUM") as ps,
        ):
            # ---------- constants ----------
            ones_sb = sb.tile([1, 128], f16, tag="onesr")
            bsp_sb = sb.tile([128, 4], f32, tag="bsp")
            bq_sb = sb.tile([128, 8], f32, tag="bq")
            bk_sb = sb.tile([128, 8], f32, tag="bk")
            bv_sb = sb.tile([1, DM], f16, tag="bv")
            nc.scalar.dma_start(out=ones_sb, in_=ones_row[:, :])
            nc.scalar.dma_start(out=bsp_sb, in_=b_sp.rearrange("(mt p) -> p mt", p=128))
            nc.scalar.dma_start(out=bq_sb, in_=bq.rearrange("(mt p) -> p mt", p=128))
            nc.scalar.dma_start(out=bk_sb, in_=bk.rearrange("(mt p) -> p mt", p=128))
            nc.scalar.dma_start(out=bv_sb, in_=bv.rearrange("(a d) -> a d", a=1))

            # stage-A prerequisites first so the PE can start ASAP
            W_sp_sb = sb.tile([128, 8 * DSP], f16, tag="two2a")        # 1MB
            nc.sync.dma_start(
                out=W_sp_sb.rearrange("p (kt m) -> p kt m", kt=8),
                in_=W_sp.rearrange("(kt p) m -> p kt m", p=128))
            mdmT_sb = sb.tile([128, KT * S], f16, tag="big4a")         # 2MB
            for half in range(2):   # split so first chains can start earlier
                nc.sync.dma_start(
                    out=mdmT_sb.rearrange("p (kt n) -> p kt n", kt=KT)[
                        :, :, half * 512:(half + 1) * 512],
                    in_=mdmT.rearrange("(kt p) n -> p kt n", p=128)[
                        :, :, half * 512:(half + 1) * 512])

            # ---------- stage A: spT = relu(W_sp^T @ mdmT + b_sp) -------
            spT_sb = sb.tile([128, 4 * S], f16, tag="two2d")
            for nch in range(2):       # k' chunk of 512 (matches mdmT DMA halves)
                for mt in range(4):    # d_sp tile
                    acc = ps.tile([128, 512], f32, tag="proj")
                    for kt in range(8):
                        nc.tensor.matmul(
                            acc,
                            W_sp_sb[:, kt * DSP + mt * 128: kt * DSP + (mt + 1) * 128],
                            mdmT_sb[:, kt * S + nch * 512: kt * S + nch * 512 + 512],
                            start=(kt == 0), stop=(kt == 7))
                    nc.vector.tensor_scalar(
                        out=spT_sb[:, mt * S + nch * 512: mt * S + nch * 512 + 512],
                        in0=acc, scalar1=bsp_sb[:, mt: mt + 1], scalar2=0.0,
                        op0=ALU.add, op1=ALU.max)

            # ---------- stage C: khT = Wk^T @ spT + bk -------------------
            Wk_sb = sb.tile([128, 4 * DM], f16, tag="two2b")
            nc.scalar.dma_start(
                out=Wk_sb.rearrange("p (ct m) -> p ct m", ct=4),
                in_=Wk.rearrange("(ct p) m -> p ct m", p=128))
            khT_sb = sb.tile([128, 8 * S], f16, tag="big4c")
            for mt in range(8):
                for nch in range(2):
                    acc = ps.tile([128, 512], f32, tag="proj")
                    for ct in range(4):
                        nc.tensor.matmul(
                            acc,
                            Wk_sb[:, ct * DM + mt * 128: ct * DM + (mt + 1) * 128],
                            spT_sb[:, ct * S + nch * 512: ct * S + nch * 512 + 512],
                            start=(ct == 0), stop=(ct == 3))
                    nc.scalar.activation(
                        khT_sb[:, mt * S + nch * 512: mt * S + nch * 512 + 512],
                        acc, AF.Identity, bias=bk_sb[:, mt: mt + 1])

            # ---------- stage D: vh (f32r, head-interleaved + ones col) --
            Wv_sb = sb.tile([128, 4 * DM], f16, tag="two2c")
            nc.scalar.dma_start(
                out=Wv_sb.rearrange("p (ct m) -> p ct m", ct=4),
                in_=Wv.rearrange("(ct p) m -> p ct m", p=128))
            vh_sb = sb.tile([128, KT * 16 * W65], f32r, tag="big4a")   # 4.2MB
            for kt in range(KT):
                for nch in range(2):
                    acc = ps.tile([128, 512], f32, tag="proj")
                    nc.tensor.matmul(
                        acc, ones_sb, bv_sb[:, nch * 512:(nch + 1) * 512],
                        start=True, stop=False)
                    for ct in range(4):
                        nc.tensor.matmul(
                            acc,
                            spT_sb[:, ct * S + kt * 128: ct * S + (kt + 1) * 128],
                            Wv_sb[:, ct * DM + nch * 512: ct * DM + nch * 512 + 512],
                            start=False, stop=(ct == 3))
                    out_view = bass.AP(
                        tensor=vh_sb.tensor,
                        offset=vh_sb.offset + kt * 16 * W65 + nch * 8 * W65,
                        ap=[vh_sb.ap[0], [W65, 8], [1, DEPTH]])
                    nc.vector.tensor_copy(
                        out_view, acc.rearrange("p (h d) -> p h d", h=8))
            ones_f32 = sb.tile([128, KT * 16], f32, tag="ones_f32")
            nc.vector.memset(ones_f32, 1.0)
            ones_view = bass.AP(
                tensor=vh_sb.tensor, offset=vh_sb.offset + DEPTH,
                ap=[vh_sb.ap[0], [W65, KT * 16], [1, 1]])
            nc.vector.tensor_copy(
                ones_view, ones_f32.rearrange("p (a b) -> p a b", b=1))

            # ---------- stage B: qhT = Wq^T @ qT + bq --------------------
            Wq_sb = sb.tile([128, 8 * DM], f16, tag="big4b")
            nc.scalar.dma_start(
                out=Wq_sb.rearrange("p (kt m) -> p kt m", kt=8),
                in_=Wq.rearrange("(kt p) m -> p kt m", p=128))
            qT_sb = sb.tile([128, 8 * Q], f16, tag="two2b")
            nc.scalar.dma_start(
                out=qT_sb.rearrange("p (kt q) -> p kt q", kt=8),
                in_=qT.rearrange("(kt p) q -> p kt q", p=128))
            qhT_sb = sb.tile([128, 8 * Q], f16, tag="two2a")
            for mt in range(8):
                acc = ps.tile([128, 512], f32, tag="proj")
                for kt in range(8):
                    nc.tensor.matmul(
                        acc,
                        Wq_sb[:, kt * DM + mt * 128: kt * DM + (mt + 1) * 128],
                        qT_sb[:, kt * Q: (kt + 1) * Q],
                        start=(kt == 0), stop=(kt == 7))
                nc.vector.tensor_scalar(
                    out=qhT_sb[:, mt * Q: (mt + 1) * Q],
                    in0=acc, scalar1=bq_sb[:, mt: mt + 1], scalar2=None,
                    op0=ALU.add)

            # ---------- stage E: per-head attention ----------------------
            mask_sb = sb.tile([128, KT * Q], f16, tag="two2c")
            nc.scalar.dma_start(
                out=mask_sb.rearrange("p (kt q) -> p kt q", kt=KT),
                in_=maskT.rearrange("(kt p) q -> p kt q", p=128))
            Wo_sb = sb.tile([128, 8 * DM], f16, tag="big4b")
            nc.scalar.dma_start(
                out=Wo_sb.rearrange("p (kt m) -> p kt m", kt=8),
                in_=Wo.rearrange("(kt p) m -> p kt m", p=128))

            concatT_sb = sb.tile([128, 8 * Q], f16, tag="two2d")
            for h in range(H):
                dmt, poff = h // 2, (h % 2) * 64
                expT = exps.tile([128, 4 * Q], f32r, tag="expT")       # half-head
                expT2 = exps.tile([128, 4 * Q], f32r, tag="expT")
                teng = nc.vector if h % 2 == 0 else nc.gpsimd
                for half, ex in ((0, expT), (1, expT2)):
                    for g in range(2):
                        lg = ps.tile([128, 1024], f32, tag="lg")
                        for j in range(2):
                            kt = half * 4 + g * 2 + j
                            nc.tensor.matmul(
                                lg[:, j * 512:(j + 1) * 512],
                                khT_sb[poff:poff + 64,
                                       dmt * S + kt * 128: dmt * S + (kt + 1) * 128],
                                qhT_sb[poff:poff + 64, dmt * Q: (dmt + 1) * Q],
                                start=True, stop=True)
                        nc.scalar.activation(
                            ex[:, g * 1024:(g + 1) * 1024], lg, AF.Exp)
                    # mask multiply (split across DVE / GPSIMD by head parity)
                    teng.tensor_tensor(
                        out=ex, in0=ex,
                        in1=mask_sb[:, half * 4 * Q: (half + 1) * 4 * Q],
                        op=ALU.mult)
                pv = ps.tile([DEPTH + 1, 512], f32, tag="pv")
                for kt in range(KT):
                    ex = expT if kt < 4 else expT2
                    nc.tensor.matmul(
                        pv,
                        vh_sb[:, kt * 16 * W65 + h * W65: kt * 16 * W65 + (h + 1) * W65],
                        ex[:, (kt % 4) * Q: (kt % 4 + 1) * Q],
                        start=(kt == 0), stop=(kt == KT - 1))
                sums_sb = small.tile([1, 512], f32, tag="sums")
                nc.scalar.copy(sums_sb, pv[DEPTH:DEPTH + 1, :])
                nc.sync.dma_start(out=sums_d[h: h + 1, :], in_=sums_sb)
                recip = small.tile([1, 512], f32, tag="recip")
                nc.vector.reciprocal_approx_fast(recip, sums_sb)
                recipB = small.tile([64, 512], f32, tag="recipB")
                nc.gpsimd.partition_broadcast(recipB, recip)
                nc.vector.tensor_tensor(
                    out=concatT_sb[poff:poff + 64, dmt * Q: (dmt + 1) * Q],
                    in0=pv[0:DEPTH, :], in1=recipB, op=ALU.mult)
                for half, ex in ((0, expT), (1, expT2)):
                    nc.sync.dma_start(
                        out=attnT_d[h, half * 512:(half + 1) * 512, :]
                            .rearrange("(kt p) q -> p kt q", p=128),
                        in_=ex.rearrange("p (kt q) -> p kt q", kt=4))

            # ---------- stage F: outT = Wo^T @ concatT -------------------
            for mt in range(8):
                acc = ps.tile([128, 512], f32, tag="proj")
                for kt in range(8):
                    nc.tensor.matmul(
                        acc,
                        Wo_sb[:, kt * DM + mt * 128: kt * DM + (mt + 1) * 128],
                        concatT_sb[:, kt * Q: (kt + 1) * Q],
                        start=(kt == 0), stop=(kt == 7))
                out_sb = small.tile([128, 512], f32, tag="outsb")
                nc.vector.tensor_copy(out_sb, acc)
                nc.sync.dma_start(
                    out=outT_d[mt * 128:(mt + 1) * 128, :], in_=out_sb)

    nc.finalize()
    return nc


_nc_cache = None


def kernel(q, min_distance_matrix, mask, W_sp, b_sp, Wq, bq, Wk, bk, Wv, bv,
           Wo, bo):
    global last_results, _nc_cache
    q = np.asarray(q, np.float32)
    mdm = np.asarray(min_distance_matrix, np.float32)
    mask = np.asarray(mask, np.float32)

    shared = {
        "W_sp": np.asarray(W_sp, np.float32).astype(np.float16),
        "Wq": (np.asarray(Wq, np.float32) * np.float32(0.125)).astype(np.float16),
        "Wk": np.asarray(Wk, np.float32).astype(np.float16),
        "Wv": np.asarray(Wv, np.float32).astype(np.float16),
        "Wo": np.asarray(Wo, np.float32).astype(np.float16),
        "b_sp": np.ascontiguousarray(b_sp, np.float32),
        "bq": np.ascontiguousarray(np.asarray(bq, np.float32) * np.float32(0.125)),
        "bk": np.ascontiguousarray(bk, np.float32),
        "bv": np.asarray(bv, np.float32).astype(np.float16),
        "ones_row": np.ones((1, 128), dtype=np.float16),
    }
    in_maps = []
    for c in range(NCORES):
        b, qc = c // 2, c % 2
        sl = slice(qc * Q, (qc + 1) * Q)
        m = dict(shared)
        m["mdmT"] = np.ascontiguousarray(mdm[b].T).astype(np.float16)
        m["qT"] = np.ascontiguousarray(q[b, sl, :].T).astype(np.float16)
        m["maskT"] = np.ascontiguousarray(1.0 - mask[b, 0, sl, :].T).astype(np.float16)
        in_maps.append(m)

    if _nc_cache is None:
        _nc_cache = _build_nc()
    res = run_bass_kernel_spmd(
        _nc_cache, in_maps, core_ids=list(range(NCORES)),
        trace=bool(os.environ.get("KERNEL_TRACE")))
    last_results = res

    out = np.empty((B, S, DM), np.float32)
    attn = np.empty((B, H, S, S), np.float32)
    bo32 = np.asarray(bo, np.float32)
    for c in range(NCORES):
        b, qc = c // 2, c % 2
        sl = slice(qc * Q, (qc + 1) * Q)
        r = res.results[c]
        out[b, sl, :] = r["outT"].T + bo32
        # attnT [H, S(k), Q] / sums [H, Q] -> [H, Q, S]
        attn[b, :, sl, :] = (r["attnT"] / r["sums"][:, None, :]).transpose(0, 2, 1)
    return out, attn


# revision 8
# speedup vs baseline: 1.8656x; 1.8656x over previous
"""TRN2 Bass kernel for nn_MultiHeadAttention_25598005084384.

Reference computation (B=4, S=1024, D_MODEL=1024, H=16, DEPTH=64, D_SP=512):
    sp   = relu(min_distance_matrix @ W_sp + b_sp)          [B,S,512]
    qh   = split_heads(q @ Wq + bq)                         [B,H,S,64]
    kh   = split_heads(sp @ Wk + bk)
    vh   = split_heads(sp @ Wv + bv)
    lg   = qh @ kh^T / 8 + mask * -1e9
    attn = softmax(lg)                                      [B,H,S,S]  (output 2)
    out  = (attn @ vh).merge_heads() @ Wo + bo              [B,S,1024] (output 1)

Sharding: 8 cores = (batch b in 0..3) x (query-chunk qc in 0..1, 512 queries).
Everything is computed in transposed ("T") layouts so the PE contraction axis
is always the SBUF partition axis and no on-device transposes are needed:
  spT[dsp,k'], khT/qhT[dm,.], vh natural [k',dm] head-interleaved with a ones
  column (so the PV matmul's row 64 yields softmax denominators), logits
  accumulated on top of a PSUM pre-primed with mask*-60000 (identity matmul),
  ACT exp evicts PSUM->SBUF, unnormalized masked exp goes out as attnT; host
  normalizes + transposes on unshard. outT = Wo^T @ concatT; host adds bo.

Precision: matmul operands in fp16 (10-bit mantissa, ~6e-4 end-to-end
scale-relative error) except the PV matmul which runs in float32r so the
attention-probability output is not further degraded. PSUM accumulates fp32.
fp16 runs at bf16 speed on the PE (1 cyc/row + fast weight load) vs 4x
slower for true fp32.
"""
import os
import sys

sys.path.insert(0, "/opt/trn_rl_repo")
import numpy as np
import concourse.bass as bass
import concourse.bacc as bacc
import concourse.tile as tile
from concourse import mybir
from concourse.bass_utils import run_bass_kernel_spmd

f32 = mybir.dt.float32
f32r = mybir.dt.float32r
f16 = mybir.dt.float16
AF = mybir.ActivationFunctionType
ALU = mybir.AluOpType

B, S, DM, H, DEPTH, DSP = 4, 1024, 1024, 16, 64, 512
Q = 512            # queries per core
NCORES = 8
KT = S // 128      # 8 key partition-tiles
W65 = DEPTH + 1    # vh head stride (64 cols + ones col)
MASKVAL = np.float32(-60000.0)   # fp16-representable; exp() underflows to 0

# stash of the last run's BassKernelResults for test harnesses
last_results = None


def _rnd_f32r(x):
    """Round-to-nearest fp32 -> 10-bit-mantissa (f32r) on host."""
    x = np.ascontiguousarray(x, np.float32)
    u = x.view(np.uint32)
    out = (((u.astype(np.uint64) + (1 << 12)) >> 13) << 13).astype(np.uint32)
    return out.view(np.float32)


def _build_nc():
    nc = bacc.Bacc(None, target_bir_lowering=False)

    # ---- DRAM I/O (per-core shard shapes) ----
    mdmT = nc.dram_tensor("mdmT", [S, S], f16, kind="ExternalInput")      # [s, k']
    qT = nc.dram_tensor("qT", [DM, Q], f16, kind="ExternalInput")         # [dm, q]
    maskT = nc.dram_tensor("maskT", [S, Q], f16, kind="ExternalInput")    # [k,q]*-6e4
    W_sp = nc.dram_tensor("W_sp", [S, DSP], f16, kind="ExternalInput")
    Wq = nc.dram_tensor("Wq", [DM, DM], f16, kind="ExternalInput")        # pre-scaled /8
    Wk = nc.dram_tensor("Wk", [DSP, DM], f16, kind="ExternalInput")
    Wv = nc.dram_tensor("Wv", [DSP, DM], f16, kind="ExternalInput")
    Wo = nc.dram_tensor("Wo", [DM, DM], f16, kind="ExternalInput")
    b_sp = nc.dram_tensor("b_sp", [DSP], f32, kind="ExternalInput")
    bq = nc.dram_tensor("bq", [DM], f32, kind="ExternalInput")            # pre-scaled /8
    bk = nc.dram_tensor("bk", [DM], f32, kind="ExternalInput")
    bv = nc.dram_tensor("bv", [DM], f16, kind="ExternalInput")
    ident = nc.dram_tensor("ident", [128, 128], f16, kind="ExternalInput")
    ones_row = nc.dram_tensor("ones_row", [1, 128], f16, kind="ExternalInput")

    attnT_d = nc.dram_tensor("attnT", [H, S, Q], f32r, kind="ExternalOutput")
    sums_d = nc.dram_tensor("sums", [H, Q], f32, kind="ExternalOutput")
    outT_d = nc.dram_tensor("outT", [DM, Q], f32, kind="ExternalOutput")

    with tile.TileContext(nc) as tc:
        with (
            tc.tile_pool(name="sb", bufs=1) as sb,
            tc.tile_pool(name="exps", bufs=3) as exps,
            tc.tile_pool(name="small", bufs=2) as small,
            tc.tile_pool(name="ps", bufs=2, space="PSUM") as ps,
        ):
            # ---------- constants ----------
            id_sb = sb.tile([128, 128], f16, tag="id")
            ones_sb = sb.tile([1, 128], f16, tag="onesr")
            bsp_sb = sb.tile([128, 4], f32, tag="bsp")
            bq_sb = sb.tile([128, 8], f32, tag="bq")
            bk_sb = sb.tile([128, 8], f32, tag="bk")
            bv_sb = sb.tile([1, DM], f16, tag="bv")
            nc.scalar.dma_start(out=id_sb, in_=ident[:, :])
            nc.scalar.dma_start(out=ones_sb, in_=ones_row[:, :])
            nc.scalar.dma_start(out=bsp_sb, in_=b_sp.rearrange("(mt p) -> p mt", p=128))
            nc.scalar.dma_start(out=bq_sb, in_=bq.rearrange("(mt p) -> p mt", p=128))
            nc.scalar.dma_start(out=bk_sb, in_=bk.rearrange("(mt p) -> p mt", p=128))
            nc.scalar.dma_start(out=bv_sb, in_=bv.rearrange("(a d) -> a d", a=1))

            # stage-A prerequisites first so the PE can start ASAP
            W_sp_sb = sb.tile([128, 8 * DSP], f16, tag="two2a")        # 1MB
            nc.sync.dma_start(
                out=W_sp_sb.rearrange("p (kt m) -> p kt m", kt=8),
                in_=W_sp.rearrange("(kt p) m -> p kt m", p=128))
            mdmT_sb = sb.tile([128, KT * S], f16, tag="big4a")         # 2MB
            for half in range(2):   # split so first chains can start earlier
                nc.sync.dma_start(
                    out=mdmT_sb.rearrange("p (kt n) -> p kt n", kt=KT)[
                        :, :, half * 512:(half + 1) * 512],
                    in_=mdmT.rearrange("(kt p) n -> p kt n", p=128)[
                        :, :, half * 512:(half + 1) * 512])

            # ---------- stage A: spT = relu(W_sp^T @ mdmT + b_sp) -------
            spT_sb = sb.tile([128, 4 * S], f16, tag="two2d")
            for nch in range(2):       # k' chunk of 512 (matches mdmT DMA halves)
                for mt in range(4):    # d_sp tile
                    acc = ps.tile([128, 512], f32, tag="proj")
                    for kt in range(8):
                        nc.tensor.matmul(
                            acc,
                            W_sp_sb[:, kt * DSP + mt * 128: kt * DSP + (mt + 1) * 128],
                            mdmT_sb[:, kt * S + nch * 512: kt * S + nch * 512 + 512],
                            start=(kt == 0), stop=(kt == 7))
                    nc.vector.tensor_scalar(
                        out=spT_sb[:, mt * S + nch * 512: mt * S + nch * 512 + 512],
                        in0=acc, scalar1=bsp_sb[:, mt: mt + 1], scalar2=0.0,
                        op0=ALU.add, op1=ALU.max)

            # ---------- stage C: khT = Wk^T @ spT + bk -------------------
            Wk_sb = sb.tile([128, 4 * DM], f16, tag="two2b")
            nc.scalar.dma_start(
                out=Wk_sb.rearrange("p (ct m) -> p ct m", ct=4),
                in_=Wk.rearrange("(ct p) m -> p ct m", p=128))
            khT_sb = sb.tile([128, 8 * S], f16, tag="big4c")
            for mt in range(8):
                for nch in range(2):
                    acc = ps.tile([128, 512], f32, tag="proj")
                    for ct in range(4):
                        nc.tensor.matmul(
                            acc,
                            Wk_sb[:, ct * DM + mt * 128: ct * DM + (mt + 1) * 128],
                            spT_sb[:, ct * S + nch * 512: ct * S + nch * 512 + 512],
                            start=(ct == 0), stop=(ct == 3))
                    nc.scalar.activation(
                        khT_sb[:, mt * S + nch * 512: mt * S + nch * 512 + 512],
                        acc, AF.Identity, bias=bk_sb[:, mt: mt + 1])

            # ---------- stage D: vh (f32r, head-interleaved + ones col) --
            Wv_sb = sb.tile([128, 4 * DM], f16, tag="two2c")
            nc.scalar.dma_start(
                out=Wv_sb.rearrange("p (ct m) -> p ct m", ct=4),
                in_=Wv.rearrange("(ct p) m -> p ct m", p=128))
            vh_sb = sb.tile([128, KT * 16 * W65], f32r, tag="big4a")   # 4.2MB
            for kt in range(KT):
                for nch in range(2):
                    acc = ps.tile([128, 512], f32, tag="proj")
                    nc.tensor.matmul(
                        acc, ones_sb, bv_sb[:, nch * 512:(nch + 1) * 512],
                        start=True, stop=False)
                    for ct in range(4):
                        nc.tensor.matmul(
                            acc,
                            spT_sb[:, ct * S + kt * 128: ct * S + (kt + 1) * 128],
                            Wv_sb[:, ct * DM + nch * 512: ct * DM + nch * 512 + 512],
                            start=False, stop=(ct == 3))
                    out_view = bass.AP(
                        tensor=vh_sb.tensor,
                        offset=vh_sb.offset + kt * 16 * W65 + nch * 8 * W65,
                        ap=[vh_sb.ap[0], [W65, 8], [1, DEPTH]])
                    nc.vector.tensor_copy(
                        out_view, acc.rearrange("p (h d) -> p h d", h=8))
            ones_f32 = sb.tile([128, KT * 16], f32, tag="ones_f32")
            nc.vector.memset(ones_f32, 1.0)
            ones_view = bass.AP(
                tensor=vh_sb.tensor, offset=vh_sb.offset + DEPTH,
                ap=[vh_sb.ap[0], [W65, KT * 16], [1, 1]])
            nc.vector.tensor_copy(
                ones_view, ones_f32.rearrange("p (a b) -> p a b", b=1))

            # ---------- stage B: qhT = Wq^T @ qT + bq --------------------
            Wq_sb = sb.tile([128, 8 * DM], f16, tag="big4b")
            nc.scalar.dma_start(
                out=Wq_sb.rearrange("p (kt m) -> p kt m", kt=8),
                in_=Wq.rearrange("(kt p) m -> p kt m", p=128))
            qT_sb = sb.tile([128, 8 * Q], f16, tag="two2b")
            nc.scalar.dma_start(
                out=qT_sb.rearrange("p (kt q) -> p kt q", kt=8),
                in_=qT.rearrange("(kt p) q -> p kt q", p=128))
            qhT_sb = sb.tile([128, 8 * Q], f16, tag="two2a")
            for mt in range(8):
                acc = ps.tile([128, 512], f32, tag="proj")
                for kt in range(8):
                    nc.tensor.matmul(
                        acc,
                        Wq_sb[:, kt * DM + mt * 128: kt * DM + (mt + 1) * 128],
                        qT_sb[:, kt * Q: (kt + 1) * Q],
                        start=(kt == 0), stop=(kt == 7))
                nc.vector.tensor_scalar(
                    out=qhT_sb[:, mt * Q: (mt + 1) * Q],
                    in0=acc, scalar1=bq_sb[:, mt: mt + 1], scalar2=None,
                    op0=ALU.add)

            # ---------- stage E: per-head attention ----------------------
            mask_sb = sb.tile([128, KT * Q], f16, tag="two2c")
            nc.scalar.dma_start(
                out=mask_sb.rearrange("p (kt q) -> p kt q", kt=KT),
                in_=maskT.rearrange("(kt p) q -> p kt q", p=128))
            Wo_sb = sb.tile([128, 8 * DM], f16, tag="big4b")
            nc.scalar.dma_start(
                out=Wo_sb.rearrange("p (kt m) -> p kt m", kt=8),
                in_=Wo.rearrange("(kt p) m -> p kt m", p=128))

            concatT_sb = sb.tile([128, 8 * Q], f16, tag="two2d")
            for h in range(H):
                dmt, poff = h // 2, (h % 2) * 64
                expT = exps.tile([128, 4 * Q], f32r, tag="expT")       # half-head
                expT2 = exps.tile([128, 4 * Q], f32r, tag="expT")
                for half, ex in ((0, expT), (1, expT2)):
                    for g in range(2):
                        lg = ps.tile([128, 1024], f32, tag="lg")
                        for j in range(2):
                            kt = half * 4 + g * 2 + j
                            nc.tensor.matmul(
                                lg[:, j * 512:(j + 1) * 512],
                                id_sb,
                                mask_sb[:, kt * Q: (kt + 1) * Q],
                                start=True, stop=False)
                            nc.tensor.matmul(
                                lg[:, j * 512:(j + 1) * 512],
                                khT_sb[poff:poff + 64,
                                       dmt * S + kt * 128: dmt * S + (kt + 1) * 128],
                                qhT_sb[poff:poff + 64, dmt * Q: (dmt + 1) * Q],
                                start=False, stop=True)
                        nc.scalar.activation(
                            ex[:, g * 1024:(g + 1) * 1024], lg, AF.Exp)
                pv = ps.tile([DEPTH + 1, 512], f32, tag="pv")
                for kt in range(KT):
                    ex = expT if kt < 4 else expT2
                    nc.tensor.matmul(
                        pv,
                        vh_sb[:, kt * 16 * W65 + h * W65: kt * 16 * W65 + (h + 1) * W65],
                        ex[:, (kt % 4) * Q: (kt % 4 + 1) * Q],
                        start=(kt == 0), stop=(kt == KT - 1))
                sums_sb = small.tile([1, 512], f32, tag="sums")
                nc.scalar.copy(sums_sb, pv[DEPTH:DEPTH + 1, :])
                nc.sync.dma_start(out=sums_d[h: h + 1, :], in_=sums_sb)
                recip = small.tile([1, 512], f32, tag="recip")
                nc.vector.reciprocal_approx_fast(recip, sums_sb)
                recipB = small.tile([64, 512], f32, tag="recipB")
                nc.gpsimd.partition_broadcast(recipB, recip)
                nc.vector.tensor_tensor(
                    out=concatT_sb[poff:poff + 64, dmt * Q: (dmt + 1) * Q],
                    in0=pv[0:DEPTH, :], in1=recipB, op=ALU.mult)
                for half, ex in ((0, expT), (1, expT2)):
                    nc.sync.dma_start(
                        out=attnT_d[h, half * 512:(half + 1) * 512, :]
                            .rearrange("(kt p) q -> p kt q", p=128),
                        in_=ex.rearrange("p (kt q) -> p kt q", kt=4))

            # ---------- stage F: outT = Wo^T @ concatT -------------------
            for mt in range(8):
                acc = ps.tile([128, 512], f32, tag="proj")
                for kt in range(8):
                    nc.tensor.matmul(
                        acc,
                        Wo_sb[:, kt * DM + mt * 128: kt * DM + (mt + 1) * 128],
                        concatT_sb[:, kt * Q: (kt + 1) * Q],
                        start=(kt == 0), stop=(kt == 7))
                out_sb = small.tile([128, 512], f32, tag="outsb")
                nc.vector.tensor_copy(out_sb, acc)
                nc.sync.dma_start(
                    out=outT_d[mt * 128:(mt + 1) * 128, :], in_=out_sb)

    nc.finalize()
    return nc


_nc_cache = None


def kernel(q, min_distance_matrix, mask, W_sp, b_sp, Wq, bq, Wk, bk, Wv, bv,
           Wo, bo):
    global last_results, _nc_cache
    q = np.asarray(q, np.float32)
    mdm = np.asarray(min_distance_matrix, np.float32)
    mask = np.asarray(mask, np.float32)

    shared = {
        "W_sp": np.asarray(W_sp, np.float32).astype(np.float16),
        "Wq": (np.asarray(Wq, np.float32) * np.float32(0.125)).astype(np.float16),
        "Wk": np.asarray(Wk, np.float32).astype(np.float16),
        "Wv": np.asarray(Wv, np.float32).astype(np.float16),
        "Wo": np.asarray(Wo, np.float32).astype(np.float16),
        "b_sp": np.ascontiguousarray(b_sp, np.float32),
        "bq": np.ascontiguousarray(np.asarray(bq, np.float32) * np.float32(0.125)),
        "bk": np.ascontiguousarray(bk, np.float32),
        "bv": np.asarray(bv, np.float32).astype(np.float16),
        "ident": np.eye(128, dtype=np.float16),
        "ones_row": np.ones((1, 128), dtype=np.float16),
    }
    in_maps = []
    for c in range(NCORES):
        b, qc = c // 2, c % 2
        sl = slice(qc * Q, (qc + 1) * Q)
        m = dict(shared)
        m["mdmT"] = np.ascontiguousarray(mdm[b].T).astype(np.float16)
        m["qT"] = np.ascontiguousarray(q[b, sl, :].T).astype(np.float16)
        m["maskT"] = np.ascontiguousarray(mask[b, 0, sl, :].T * MASKVAL).astype(np.float16)
        in_maps.append(m)

    if _nc_cache is None:
        _nc_cache = _build_nc()
    res = run_bass_kernel_spmd(
        _nc_cache, in_maps, core_ids=list(range(NCORES)),
        trace=bool(os.environ.get("KERNEL_TRACE")))
    last_results = res

    out = np.empty((B, S, DM), np.float32)
    attn = np.empty((B, H, S, S), np.float32)
    bo32 = np.asarray(bo, np.float32)
    for c in range(NCORES):
        b, qc = c // 2, c % 2
        sl = slice(qc * Q, (qc + 1) * Q)
        r = res.results[c]
        out[b, sl, :] = r["outT"].T + bo32
        # attnT [H, S(k), Q] / sums [H, Q] -> [H, Q, S]
        attn[b, :, sl, :] = (r["attnT"] / r["sums"][:, None, :]).transpose(0, 2, 1)
    return out, attn


# revision 10
# speedup vs baseline: 2.0029x; 1.0736x over previous
"""TRN2 Bass kernel for nn_MultiHeadAttention_25598005084384.

Reference computation (B=4, S=1024, D_MODEL=1024, H=16, DEPTH=64, D_SP=512):
    sp   = relu(min_distance_matrix @ W_sp + b_sp)          [B,S,512]
    qh   = split_heads(q @ Wq + bq)                         [B,H,S,64]
    kh   = split_heads(sp @ Wk + bk)
    vh   = split_heads(sp @ Wv + bv)
    lg   = qh @ kh^T / 8 + mask * -1e9
    attn = softmax(lg)                                      [B,H,S,S]  (output 2)
    out  = (attn @ vh).merge_heads() @ Wo + bo              [B,S,1024] (output 1)

Sharding: 8 cores = (batch b in 0..3) x (query-chunk qc in 0..1, 512 queries).
Everything is computed in transposed ("T") layouts so the PE contraction axis
is always the SBUF partition axis and no on-device transposes are needed:
  spT[dsp,k'], khT/qhT[dm,.], vh natural [k',dm] head-interleaved with a ones
  column (so the PV matmul's row 64 yields softmax denominators), logits
  accumulated on top of a PSUM pre-primed with mask*-60000 (identity matmul),
  ACT exp evicts PSUM->SBUF, unnormalized masked exp goes out as attnT; host
  normalizes + transposes on unshard. outT = Wo^T @ concatT; host adds bo.

Precision: matmul operands in fp16 (10-bit mantissa, ~6e-4 end-to-end
scale-relative error) except the PV matmul which runs in float32r so the
attention-probability output is not further degraded. PSUM accumulates fp32.
fp16 runs at bf16 speed on the PE (1 cyc/row + fast weight load) vs 4x
slower for true fp32.
"""
import os
import sys

sys.path.insert(0, "/opt/trn_rl_repo")
import numpy as np
import concourse.bass as bass
import concourse.bacc as bacc
import concourse.tile as tile
from concourse import mybir
from concourse.bass_utils import run_bass_kernel_spmd

f32 = mybir.dt.float32
f32r = mybir.dt.float32r
f16 = mybir.dt.float16
AF = mybir.ActivationFunctionType
ALU = mybir.AluOpType

B, S, DM, H, DEPTH, DSP = 4, 1024, 1024, 16, 64, 512
Q = 512            # queries per core
NCORES = 8
KT = S // 128      # 8 key partition-tiles
W65 = DEPTH + 1    # vh head stride (64 cols + ones col)
MASKVAL = np.float32(-60000.0)   # fp16-representable; exp() underflows to 0

# stash of the last run's BassKernelResults for test harnesses
last_results = None


def _rnd_f32r(x):
    """Round-to-nearest fp32 -> 10-bit-mantissa (f32r) on host."""
    x = np.ascontiguousarray(x, np.float32)
    u = x.view(np.uint32)
    out = (((u.astype(np.uint64) + (1 << 12)) >> 13) << 13).astype(np.uint32)
    return out.view(np.float32)


def _build_nc():
    nc = bacc.Bacc(None, target_bir_lowering=False)

    # ---- DRAM I/O (per-core shard shapes) ----
    mdmT = nc.dram_tensor("mdmT", [S, S], f16, kind="ExternalInput")      # [s, k']
    qT = nc.dram_tensor("qT", [DM, Q], f16, kind="ExternalInput")         # [dm, q]
    maskT = nc.dram_tensor("maskT", [S, Q], f16, kind="ExternalInput")    # [k,q]*-6e4
    W_sp = nc.dram_tensor("W_sp", [S, DSP], f16, kind="ExternalInput")
    Wq = nc.dram_tensor("Wq", [DM, DM], f16, kind="ExternalInput")        # pre-scaled /8
    Wk = nc.dram_tensor("Wk", [DSP, DM], f16, kind="ExternalInput")
    Wv = nc.dram_tensor("Wv", [DSP, DM], f16, kind="ExternalInput")
    Wo = nc.dram_tensor("Wo", [DM, DM], f16, kind="ExternalInput")
    b_sp = nc.dram_tensor("b_sp", [DSP], f32, kind="ExternalInput")
    bq = nc.dram_tensor("bq", [DM], f32, kind="ExternalInput")            # pre-scaled /8
    bk = nc.dram_tensor("bk", [DM], f32, kind="ExternalInput")
    bv = nc.dram_tensor("bv", [DM], f16, kind="ExternalInput")
    ident = nc.dram_tensor("ident", [128, 128], f16, kind="ExternalInput")
    ones_row = nc.dram_tensor("ones_row", [1, 128], f16, kind="ExternalInput")

    attnT_d = nc.dram_tensor("attnT", [H, S, Q], f32r, kind="ExternalOutput")
    sums_d = nc.dram_tensor("sums", [H, Q], f32, kind="ExternalOutput")
    outT_d = nc.dram_tensor("outT", [DM, Q], f32, kind="ExternalOutput")

    with tile.TileContext(nc) as tc:
        with (
            tc.tile_pool(name="sb", bufs=1) as sb,
            tc.tile_pool(name="exps", bufs=3) as exps,
            tc.tile_pool(name="small", bufs=2) as small,
            tc.tile_pool(name="ps", bufs=2, space="PSUM") as ps,
        ):
            # ---------- constants ----------
            id_sb = sb.tile([128, 128], f16, tag="id")
            ones_sb = sb.tile([1, 128], f16, tag="onesr")
            bsp_sb = sb.tile([128, 4], f32, tag="bsp")
            bq_sb = sb.tile([128, 8], f32, tag="bq")
            bk_sb = sb.tile([128, 8], f32, tag="bk")
            bv_sb = sb.tile([1, DM], f16, tag="bv")
            nc.scalar.dma_start(out=id_sb, in_=ident[:, :])
            nc.scalar.dma_start(out=ones_sb, in_=ones_row[:, :])
            nc.scalar.dma_start(out=bsp_sb, in_=b_sp.rearrange("(mt p) -> p mt", p=128))
            nc.scalar.dma_start(out=bq_sb, in_=bq.rearrange("(mt p) -> p mt", p=128))
            nc.scalar.dma_start(out=bk_sb, in_=bk.rearrange("(mt p) -> p mt", p=128))
            nc.scalar.dma_start(out=bv_sb, in_=bv.rearrange("(a d) -> a d", a=1))

            # stage-A prerequisites first so the PE can start ASAP
            W_sp_sb = sb.tile([128, 8 * DSP], f16, tag="two2a")        # 1MB
            mdmT_sb = sb.tile([128, KT * S], f16, tag="big4a")         # 2MB
            Wsp_r = W_sp.rearrange("(kt p) m -> p kt m", p=128)
            Wsp_o = W_sp_sb.rearrange("p (kt m) -> p kt m", kt=8)
            mdm_r = mdmT.rearrange("(kt p) n -> p kt n", p=128)
            mdm_o = mdmT_sb.rearrange("p (kt n) -> p kt n", kt=KT)
            nc.sync.dma_start(out=Wsp_o[:, 0:4], in_=Wsp_r[:, 0:4])
            nc.sync.dma_start(out=mdm_o[:, 0:4, 0:512], in_=mdm_r[:, 0:4, 0:512])
            nc.sync.dma_start(out=Wsp_o[:, 4:8], in_=Wsp_r[:, 4:8])
            nc.sync.dma_start(out=mdm_o[:, 4:8, 0:512], in_=mdm_r[:, 4:8, 0:512])
            nc.sync.dma_start(out=mdm_o[:, :, 512:1024], in_=mdm_r[:, :, 512:1024])

            Wq_sb = sb.tile([128, 8 * DM], f16, tag="big4b")
            nc.sync.dma_start(
                out=Wq_sb.rearrange("p (kt m) -> p kt m", kt=8),
                in_=Wq.rearrange("(kt p) m -> p kt m", p=128))
            qT_sb = sb.tile([128, 8 * Q], f16, tag="qTt")
            nc.sync.dma_start(
                out=qT_sb.rearrange("p (kt q) -> p kt q", kt=8),
                in_=qT.rearrange("(kt p) q -> p kt q", p=128))

            # ---------- stage A: spT = relu(W_sp^T @ mdmT + b_sp) -------
            spT_sb = sb.tile([128, 4 * S], f16, tag="two2d")
            for nch in range(2):       # k' chunk of 512 (matches mdmT DMA halves)
                for mt in range(4):    # d_sp tile
                    acc = ps.tile([128, 512], f32, tag="proj")
                    for kt in range(8):
                        nc.tensor.matmul(
                            acc,
                            W_sp_sb[:, kt * DSP + mt * 128: kt * DSP + (mt + 1) * 128],
                            mdmT_sb[:, kt * S + nch * 512: kt * S + nch * 512 + 512],
                            start=(kt == 0), stop=(kt == 7))
                    nc.vector.tensor_scalar(
                        out=spT_sb[:, mt * S + nch * 512: mt * S + nch * 512 + 512],
                        in0=acc, scalar1=bsp_sb[:, mt: mt + 1], scalar2=0.0,
                        op0=ALU.add, op1=ALU.max)

            # ---------- stage C: khT = Wk^T @ spT + bk -------------------
            Wk_sb = sb.tile([128, 4 * DM], f16, tag="two2b")
            nc.scalar.dma_start(
                out=Wk_sb.rearrange("p (ct m) -> p ct m", ct=4),
                in_=Wk.rearrange("(ct p) m -> p ct m", p=128))
            khT_sb = sb.tile([128, 8 * S], f16, tag="big4c")
            for mt in range(8):
                for nch in range(2):
                    acc = ps.tile([128, 512], f32, tag="proj")
                    for ct in range(4):
                        nc.tensor.matmul(
                            acc,
                            Wk_sb[:, ct * DM + mt * 128: ct * DM + (mt + 1) * 128],
                            spT_sb[:, ct * S + nch * 512: ct * S + nch * 512 + 512],
                            start=(ct == 0), stop=(ct == 3))
                    nc.scalar.activation(
                        khT_sb[:, mt * S + nch * 512: mt * S + nch * 512 + 512],
                        acc, AF.Identity, bias=bk_sb[:, mt: mt + 1])

            # ---------- stage D: vh (f32r, head-interleaved + ones col) --
            Wv_sb = sb.tile([128, 4 * DM], f16, tag="two2c")
            nc.scalar.dma_start(
                out=Wv_sb.rearrange("p (ct m) -> p ct m", ct=4),
                in_=Wv.rearrange("(ct p) m -> p ct m", p=128))
            vh_sb = sb.tile([128, KT * 16 * W65], f32r, tag="big4a")   # 4.2MB
            for kt in range(KT):
                for nch in range(2):
                    acc = ps.tile([128, 512], f32, tag="proj")
                    nc.tensor.matmul(
                        acc, ones_sb, bv_sb[:, nch * 512:(nch + 1) * 512],
                        start=True, stop=False)
                    for ct in range(4):
                        nc.tensor.matmul(
                            acc,
                            spT_sb[:, ct * S + kt * 128: ct * S + (kt + 1) * 128],
                            Wv_sb[:, ct * DM + nch * 512: ct * DM + nch * 512 + 512],
                            start=False, stop=(ct == 3))
                    out_view = bass.AP(
                        tensor=vh_sb.tensor,
                        offset=vh_sb.offset + kt * 16 * W65 + nch * 8 * W65,
                        ap=[vh_sb.ap[0], [W65, 8], [1, DEPTH]])
                    nc.vector.tensor_copy(
                        out_view, acc.rearrange("p (h d) -> p h d", h=8))
            ones_f32 = sb.tile([128, KT * 16], f32, tag="ones_f32")
            nc.vector.memset(ones_f32, 1.0)
            ones_view = bass.AP(
                tensor=vh_sb.tensor, offset=vh_sb.offset + DEPTH,
                ap=[vh_sb.ap[0], [W65, KT * 16], [1, 1]])
            nc.vector.tensor_copy(
                ones_view, ones_f32.rearrange("p (a b) -> p a b", b=1))

            # ---------- stage B: qhT (zero-padded per head for K=128 FWL) -
            # layout: [128, 16 heads * 512]; head h data in rows (h%2)*64..+64,
            # complementary 64 rows are zero so logits can contract K=128
            # against the full khT tile (other head's rows hit zeros).
            qhT_sb = sb.tile([128, H * Q], f16, tag="two2a")
            nc.vector.memset(qhT_sb, 0.0)
            for mt in range(8):
                acc = ps.tile([128, 512], f32, tag="proj")
                for kt in range(8):
                    nc.tensor.matmul(
                        acc,
                        Wq_sb[:, kt * DM + mt * 128: kt * DM + (mt + 1) * 128],
                        qT_sb[:, kt * Q: (kt + 1) * Q],
                        start=(kt == 0), stop=(kt == 7))
                nc.vector.tensor_scalar(
                    out=qhT_sb[0:64, 2 * mt * Q: (2 * mt + 1) * Q],
                    in0=acc[0:64, :], scalar1=bq_sb[0:64, mt: mt + 1],
                    scalar2=None, op0=ALU.add)
                nc.vector.tensor_scalar(
                    out=qhT_sb[64:128, (2 * mt + 1) * Q: (2 * mt + 2) * Q],
                    in0=acc[64:128, :], scalar1=bq_sb[64:128, mt: mt + 1],
                    scalar2=None, op0=ALU.add)

            # ---------- stage E: per-head attention ----------------------
            mask_sb = sb.tile([128, KT * Q], f16, tag="two2c")
            nc.scalar.dma_start(
                out=mask_sb.rearrange("p (kt q) -> p kt q", kt=KT),
                in_=maskT.rearrange("(kt p) q -> p kt q", p=128))
            Wo_sb = sb.tile([128, 8 * DM], f16, tag="big4b")
            nc.scalar.dma_start(
                out=Wo_sb.rearrange("p (kt m) -> p kt m", kt=8),
                in_=Wo.rearrange("(kt p) m -> p kt m", p=128))

            concatT_sb = sb.tile([128, 8 * Q], f16, tag="two2d")
            for h in range(H):
                dmt, poff = h // 2, (h % 2) * 64
                expT = exps.tile([128, 4 * Q], f32r, tag="expT")       # half-head
                expT2 = exps.tile([128, 4 * Q], f32r, tag="expT")
                for half, ex in ((0, expT), (1, expT2)):
                    for g in range(2):
                        lg = ps.tile([128, 1024], f32, tag="lg")
                        for j in range(2):
                            kt = half * 4 + g * 2 + j
                            nc.tensor.matmul(
                                lg[:, j * 512:(j + 1) * 512],
                                id_sb,
                                mask_sb[:, kt * Q: (kt + 1) * Q],
                                start=True, stop=False)
                            nc.tensor.matmul(
                                lg[:, j * 512:(j + 1) * 512],
                                khT_sb[:, dmt * S + kt * 128: dmt * S + (kt + 1) * 128],
                                qhT_sb[:, h * Q: (h + 1) * Q],
                                start=False, stop=True)
                        nc.scalar.activation(
                            ex[:, g * 1024:(g + 1) * 1024], lg, AF.Exp)
                pv = ps.tile([DEPTH + 1, 512], f32, tag="pv")
                for kt in range(KT):
                    ex = expT if kt < 4 else expT2
                    nc.tensor.matmul(
                        pv,
                        vh_sb[:, kt * 16 * W65 + h * W65: kt * 16 * W65 + (h + 1) * W65],
                        ex[:, (kt % 4) * Q: (kt % 4 + 1) * Q],
                        start=(kt == 0), stop=(kt == KT - 1))
                sums_sb = small.tile([1, 512], f32, tag="sums")
                nc.scalar.copy(sums_sb, pv[DEPTH:DEPTH + 1, :])
                nc.sync.dma_start(out=sums_d[h: h + 1, :], in_=sums_sb)
                recip = small.tile([1, 512], f32, tag="recip")
                nc.vector.reciprocal_approx_fast(recip, sums_sb)
                recipB = small.tile([64, 512], f32, tag="recipB")
                nc.gpsimd.partition_broadcast(recipB, recip)
                nc.vector.tensor_tensor(
                    out=concatT_sb[poff:poff + 64, dmt * Q: (dmt + 1) * Q],
                    in0=pv[0:DEPTH, :], in1=recipB, op=ALU.mult)
                for half, ex in ((0, expT), (1, expT2)):
                    nc.sync.dma_start(
                        out=attnT_d[h, half * 512:(half + 1) * 512, :]
                            .rearrange("(kt p) q -> p kt q", p=128),
                        in_=ex.rearrange("p (kt q) -> p kt q", kt=4))

            # ---------- stage F: outT = Wo^T @ concatT -------------------
            for mt in range(8):
                acc = ps.tile([128, 512], f32, tag="proj")
                for kt in range(8):
                    nc.tensor.matmul(
                        acc,
                        Wo_sb[:, kt * DM + mt * 128: kt * DM + (mt + 1) * 128],
                        concatT_sb[:, kt * Q: (kt + 1) * Q],
                        start=(kt == 0), stop=(kt == 7))
                out_sb = small.tile([128, 512], f32, tag="outsb")
                nc.vector.tensor_copy(out_sb, acc)
                nc.sync.dma_start(
                    out=outT_d[mt * 128:(mt + 1) * 128, :], in_=out_sb)

    nc.finalize()
    return nc


_nc_cache = None


def kernel(q, min_distance_matrix, mask, W_sp, b_sp, Wq, bq, Wk, bk, Wv, bv,
           Wo, bo):
    global last_results, _nc_cache
    q = np.asarray(q, np.float32)
    mdm = np.asarray(min_distance_matrix, np.float32)
    mask = np.asarray(mask, np.float32)

    shared = {
        "W_sp": np.asarray(W_sp, np.float32).astype(np.float16),
        "Wq": (np.asarray(Wq, np.float32) * np.float32(0.125)).astype(np.float16),
        "Wk": np.asarray(Wk, np.float32).astype(np.float16),
        "Wv": np.asarray(Wv, np.float32).astype(np.float16),
        "Wo": np.asarray(Wo, np.float32).astype(np.float16),
        "b_sp": np.ascontiguousarray(b_sp, np.float32),
        "bq": np.ascontiguousarray(np.asarray(bq, np.float32) * np.float32(0.125)),
        "bk": np.ascontiguousarray(bk, np.float32),
        "bv": np.asarray(bv, np.float32).astype(np.float16),
        "ident": np.eye(128, dtype=np.float16),
        "ones_row": np.ones((1, 128), dtype=np.float16),
    }
    in_maps = []
    for c in range(NCORES):
        b, qc = c // 2, c % 2
        sl = slice(qc * Q, (qc + 1) * Q)
        m = dict(shared)
        m["mdmT"] = np.ascontiguousarray(mdm[b].T).astype(np.float16)
        m["qT"] = np.ascontiguousarray(q[b, sl, :].T).astype(np.float16)
        m["maskT"] = np.ascontiguousarray(mask[b, 0, sl, :].T * MASKVAL).astype(np.float16)
        in_maps.append(m)

    if _nc_cache is None:
        _nc_cache = _build_nc()
    res = run_bass_kernel_spmd(
        _nc_cache, in_maps, core_ids=list(range(NCORES)),
        trace=bool(os.environ.get("KERNEL_TRACE")))
    last_results = res

    out = np.empty((B, S, DM), np.float32)
    attn = np.empty((B, H, S, S), np.float32)
    bo32 = np.asarray(bo, np.float32)
    for c in range(NCORES):
        b, qc = c // 2, c % 2
        sl = slice(qc * Q, (qc + 1) * Q)
        r = res.results[c]
        out[b, sl, :] = r["outT"].T + bo32
        # attnT [H, S(k), Q] / sums [H, Q] -> [H, Q, S]
        attn[b, :, sl, :] = (r["attnT"] / r["sums"][:, None, :]).transpose(0, 2, 1)
    return out, attn
